# revision 29
# baseline (speedup 1.0000x reference)
"""Trainium2 Bass kernel for AttentionWithRotaryPosEmb (8 cores, data-parallel).

Strategy
--------
Data-parallel over batch: each of the 8 NeuronCores computes one batch element
end-to-end. No collectives needed.

Per-core pipeline (batch x_b is [C=256, S=1024]):
  1. QKV projection:  q,k as [o, s] (o = 64h+d), vT as [s, hid] -- the vT
     orientation comes straight out of the matmul (lhsT=x, rhs=w_v^T), so no
     transposes are ever needed on-chip.  PSUM->SBUF casts run on the (idle)
     Pool/GPSIMD engine so DVE stays free for rope math.
  2. RoPE on q,k in bf16 [d, s] layout (DVE 2x packed mode): rotate-half is a
     partition-shifted DMA copy into a scratch tile, then sin/cos multiplies
     against fp16 row tables and one add.
  3. L2 norm over the sequence axis: fused square+reduce on DVE
     (tensor_tensor_reduce), rsqrt via exp(-0.5*ln(x)) on ACT, both q and k
     scales folded into q with one tensor_scalar multiply.
  4. Attention with *transposed* softmax: simT[j, i] blocks via
     matmul(lhsT=k_h, rhs=q_h); exp on ACT with scale=10 applied for free; the
     softmax denominators come for free from a ones-column appended to vT
     (65th lhsT column); outT[d, i] accumulates over j-blocks in PSUM.
     No max-subtraction needed: |10*sim| <= ~1 by construction (l2-normalized).
  5. Normalize via gpsimd partition_broadcast of the denominator row + one
     fused DVE divide (PSUM -> bf16), then the output projection and bias.

Engine budget per core (cost-model ns): PE 68.2us (320 matmuls), ACT ~67us
(64 softmax exps), DVE ~46us, Pool ~31us, all overlapped.
"""

import sys

import numpy as np

if "/opt/trn_rl_repo" not in sys.path:
    sys.path.insert(0, "/opt/trn_rl_repo")

HEADS = 8
DH = 64
S = 1024
C = 256
HID = 512
ROT = 32
HALF = 16
SCALE = 10.0
N_CORES = 8

_CACHE = {}


def _rope_tables():
    """Row-patterned cos/sin tables [128, S] matching the q/k SBUF layout.

    Partition row r holds o-row (128t + r) of q/k tile t, i.e. head 2t + (r>=64)
    and d = r % 64.  Rows d in [0,16) get cos(i*invf[d]) / -sin(i*invf[d]);
    rows d in [16,32) get cos(i*invf[d-16]) / +sin(i*invf[d-16]); rows d >= 32
    get cos=1, sin=0 (identity).  The sign of sin encodes rotate_half.
    fp16 keeps the DVE multiplies in 2x packed mode (f32 tables would drop
    the whole rope pipeline to full-rate).
    """
    inv = (
        1.0 / (np.float32(10000.0) ** (np.arange(0, ROT, 2, dtype=np.float32) / np.float32(ROT)))
    ).astype(np.float32)
    ang = (np.arange(S, dtype=np.float32)[None, :] * inv[:, None]).astype(np.float32)
    cos16 = np.cos(ang).astype(np.float32)  # [16, S]
    sin16 = np.sin(ang).astype(np.float32)
    cosT = np.ones((128, S), np.float32)
    sinT = np.zeros((128, S), np.float32)
    for r in range(128):
        d = r % 64
        if d < HALF:
            cosT[r] = cos16[d]
            sinT[r] = -sin16[d]
        elif d < ROT:
            cosT[r] = cos16[d - HALF]
            sinT[r] = sin16[d - HALF]
    return cosT.astype(np.float16), sinT.astype(np.float16)


def _emit(ctx, tc, aps):
    import concourse.bass as bass  # noqa: F401
    from concourse import mybir

    f32 = mybir.dt.float32
    bf16 = mybir.dt.bfloat16
    f16 = mybir.dt.float16
    AF = mybir.ActivationFunctionType
    ALU = mybir.AluOpType
    nc = tc.nc
    x_d, wq_d, wo_d, bo_d, cos_d, sin_d, zer_d, out_d = aps

    singles = ctx.enter_context(tc.tile_pool(name="singles", bufs=1))
    etp = ctx.enter_context(tc.tile_pool(name="etp", bufs=12))
    ppm = ctx.enter_context(tc.tile_pool(name="ppm", bufs=2, space="PSUM"))
    ppo = ctx.enter_context(tc.tile_pool(name="ppo", bufs=2, space="PSUM"))

    # ---- persistent SBUF ----
    # matmul operands are bf16 (rounded once, late); rope math in bf16 too
    # (DVE 2x packed mode), norm stats and softmax denominators in f32
    sb_x = singles.tile([128, 2, S], bf16)
    sb_wq = singles.tile([128, 2, 3 * HID], bf16)
    sb_wo = singles.tile([128, 4, C], bf16)
    sb_bo = singles.tile([1, 256], bf16)
    sb_one = singles.tile([1, S], bf16)
    sb_cos = singles.tile([128, S], f16)
    sb_sin = singles.tile([128, S], f16)
    sb_qr = singles.tile([128, 4, S], bf16)   # raw q (pre-rope), from PSUM
    sb_kr = singles.tile([128, 4, S], bf16)
    sb_tq = singles.tile([128, 4, S], bf16)   # rotate-half scratch
    sb_tk = singles.tile([128, 4, S], bf16)
    sb_qb = singles.tile([128, 4, S], bf16)   # roped+normalized q (matmul op)
    sb_kb = singles.tile([128, 4, S], bf16)   # roped k (matmul operand)
    sb_vx = singles.tile([128, 8, HEADS * (DH + 1)], bf16)
    sb_R = singles.tile([64, 8, S], f32)      # broadcast softmax denominators
    sb_rc = singles.tile([1, 8, S], f32)      # per-head reciprocal rows
    sb_o = singles.tile([128, 4, S], bf16)    # attention out (matmul operand)
    sb_stat = singles.tile([128, 16], f32)

    # ---- input loads (c-block granularity so QKV matmuls can start early) ----
    # SP queue carries only the startup-critical loads; wo/bias/zeros go on
    # the (idle-at-start) ACT queue so they don't delay the rope rotate DMAs.
    nc.sync.dma_start(out=sb_wq[:, 0, :], in_=wq_d[0:128, :])
    nc.sync.dma_start(out=sb_x[:, 0, :], in_=x_d[0:128, :])
    nc.sync.dma_start(out=sb_wq[:, 1, :], in_=wq_d[128:256, :])
    nc.sync.dma_start(out=sb_x[:, 1, :], in_=x_d[128:256, :])
    nc.sync.dma_start(out=sb_cos, in_=cos_d[:, :])
    nc.sync.dma_start(out=sb_sin, in_=sin_d[:, :])
    nc.vector.memset(sb_one[:, :], 1.0)

    def load_weights_late():
        """wo/bias loads, emitted after the first exp burst: they are only
        needed ~90us in, and issuing them at t=0 would steal the HWDGE
        (one shared 625ns/DMA generator) from the startup-critical loads."""
        for kk in range(4):
            nc.scalar.dma_start(
                out=sb_wo[:, kk, :], in_=wo_d[kk * 128 : (kk + 1) * 128, :]
            )
        nc.scalar.dma_start(out=sb_bo[:, :], in_=bo_d[:, :])

    # ones column per head in vT_ext -> free softmax denominators
    vx4 = sb_vx.rearrange("p j (h e) -> p j h e", e=DH + 1)
    nc.vector.memset(vx4[:, :, :, DH : DH + 1], 1.0)

    def copy_ps(eng, out, in_):
        """PSUM f32 -> SBUF bf16 cast on the named engine queue."""
        if eng == "act":
            nc.scalar.copy(out=out, in_=in_)
        elif eng == "dve":
            nc.vector.tensor_copy(out=out, in_=in_)
        else:
            nc.gpsimd.tensor_copy(out=out, in_=in_)

    def qk_block(ob, pool, cp):
        """QKV matmul for q (ob<4) / k (ob>=4) o-block -> SBUF bf16 cast.

        `pool` picks the PSUM rotation (ppm shares the sim-block tag, ppo
        the v/attention-out tag) and `cp` the copy-engine queue; both are
        chosen per block so no pool rotation ever waits on a slow copy."""
        tag = "mm" if pool is ppm else "ov"
        ps = pool.tile([128, S], f32, tag=tag, name=f"ps_qk{ob}")
        for nn in range(2):
            for kk in range(2):
                nc.tensor.matmul(
                    ps[:, nn * 512 : (nn + 1) * 512],
                    lhsT=(sb_wq[:, kk, ob * 128 : (ob + 1) * 128]),
                    rhs=(sb_x[:, kk, nn * 512 : (nn + 1) * 512]),
                    start=(kk == 0),
                    stop=(kk == 1),
                )
        dst3 = sb_qr if ob < 4 else sb_kr
        copy_ps(cp, dst3[:, ob % 4, :], ps[:, :])

    def rope_core(t):
        """RoPE for q/k tile t: rotate-half swaps + sin/cos multiply-add and
        the fused sum-of-squares.  q-side swaps ride the SP hardware DGE,
        k-side swaps run as Pool-engine software DGE so the two dispatch
        chains proceed in parallel.  DVE ops are interleaved q/k so neither
        side's rotate latency blocks the other's cos multiply."""
        q, k = sb_qr[:, t, :], sb_kr[:, t, :]
        for src3, tmp3, dq in ((sb_qr, sb_tq, nc.sync), (sb_kr, sb_tk, nc.gpsimd)):
            for base in (0, 64):
                dq.dma_start(
                    out=tmp3[base : base + 16, t, :],
                    in_=src3[base + 16 : base + 32, t, :],
                )
                dq.dma_start(
                    out=tmp3[base + 16 : base + 32, t, :],
                    in_=src3[base : base + 16, t, :],
                )
        tq, tk = sb_tq[:, t, :], sb_tk[:, t, :]
        nc.vector.tensor_mul(out=q, in0=q, in1=sb_cos[:, :])
        nc.vector.tensor_mul(out=k, in0=k, in1=sb_cos[:, :])
        # sign folded into sinT; its zero rows kill the d>=32 garbage
        nc.vector.tensor_mul(out=tq, in0=tq, in1=sb_sin[:, :])
        nc.vector.tensor_mul(out=tk, in0=tk, in1=sb_sin[:, :])
        nc.vector.tensor_add(out=sb_qb[:, t, :], in0=q, in1=tq)
        nc.vector.tensor_add(out=sb_kb[:, t, :], in0=k, in1=tk)
        for dst3, tmp3, col in ((sb_qb, sb_tq, 0), (sb_kb, sb_tk, 4)):
            # fused sum-of-squares over s; the full-width product is dumped
            # into the (now dead) rotate scratch
            nc.vector.tensor_tensor_reduce(
                out=tmp3[:, t, :],
                in0=dst3[:, t, :],
                in1=dst3[:, t, :],
                scale=1.0,
                scalar=0.0,
                op0=ALU.mult,
                op1=ALU.add,
                accum_out=sb_stat[:, col + t : col + t + 1],
            )

    def rope_finish(t):
        """rs_comb = (ssq_q*ssq_k)^-1/2 on ACT, folded into q on DVE."""
        nc.scalar.mul(
            out=sb_stat[:, 8 + t : 9 + t],
            in_=sb_stat[:, t : t + 1],
            mul=sb_stat[:, 4 + t : 5 + t],
        )
        nc.scalar.activation(
            out=sb_stat[:, 8 + t : 9 + t], in_=sb_stat[:, 8 + t : 9 + t],
            func=AF.Ln, bias=0.0,
        )
        nc.scalar.activation(
            out=sb_stat[:, 12 + t : 13 + t], in_=sb_stat[:, 8 + t : 9 + t],
            func=AF.Exp, scale=-0.5,
        )
        nc.vector.tensor_scalar_mul(
            out=sb_qb[:, t, :], in0=sb_qb[:, t, :],
            scalar1=sb_stat[:, 12 + t : 13 + t],
        )

    def v_block(jb):
        """vT s-block: [s_jb, hid] straight from matmul, strided into vx ext."""
        psv = ppo.tile([128, 512], f32, tag="ov", name=f"psv{jb}")
        for kk in range(2):
            nc.tensor.matmul(
                psv[:, :],
                lhsT=(sb_x[:, kk, jb * 128 : (jb + 1) * 128]),
                rhs=(sb_wq[:, kk, 2 * HID : 3 * HID]),
                start=(kk == 0),
                stop=(kk == 1),
            )
        dst = sb_vx[:, jb, :].rearrange("p (h e) -> p h e", e=DH + 1)[:, :, 0:DH]
        src = psv.rearrange("p (h d) -> p h d", d=DH)
        # f32 PSUM -> bf16 SBUF cast; Pool cannot read PSUM, so these all
        # ride the (pre-softmax idle) ACT queue
        copy_ps("act", dst, src)

    def attn_sims(h):
        """sim blocks + exps for head h; returns the et tiles for attn_avs."""
        t, half = h // 2, h % 2
        b0 = 64 * half
        q_h = sb_qb[b0 : b0 + 64, t, :]
        k_h = sb_kb[b0 : b0 + 64, t, :]
        ets = []
        for jb in range(8):
            pss = ppm.tile([128, S], f32, tag="mm", name=f"pss{h}_{jb}")
            for nn in range(2):
                nc.tensor.matmul(
                    pss[:, nn * 512 : (nn + 1) * 512],
                    lhsT=(k_h[:, jb * 128 : (jb + 1) * 128]),
                    rhs=(q_h[:, nn * 512 : (nn + 1) * 512]),
                    start=True,
                    stop=True,
                )
            et = etp.tile([128, S], bf16, tag="et", name=f"et{h}_{jb}", bufs=12)
            nc.scalar.activation(out=et[:, :], in_=pss[:, :], func=AF.Exp, scale=SCALE)
            ets.append(et)
        return ets

    def attn_avs(h, ets):
        """av accumulation + softmax normalization for head h."""
        t, half = h // 2, h % 2
        b0 = 64 * half
        pso = ppo.tile([65, S], f32, tag="ov", name=f"pso{h}")
        for jb in range(8):
            for nn in range(2):
                nc.tensor.matmul(
                    pso[:, nn * 512 : (nn + 1) * 512],
                    lhsT=(sb_vx[:, jb, h * (DH + 1) : (h + 1) * (DH + 1)]),
                    rhs=(ets[jb][:, nn * 512 : (nn + 1) * 512]),
                    start=(jb == 0),
                    stop=(jb == 7),
                )
        # row 64 of pso = sum_j exp(sim) per i.  Pool cannot read PSUM, so:
        # reciprocal on DVE (PSUM -> SBUF row), partition-broadcast of the
        # SBUF row on Pool, then one DVE multiply normalizes the head.
        # The broadcast output stays at base partition 0 (per-head slots):
        # GPSIMD writes to a nonzero base partition are broken on HW.
        rcp = sb_rc[0:1, h, :]
        nc.vector.reciprocal(out=rcp, in_=pso[64:65, :])
        nc.sync.dma_start(out=sb_R[0:1, h, :], in_=rcp)
        w = 1
        while w < 64:
            nc.sync.dma_start(
                out=sb_R[w : 2 * w, h, :], in_=sb_R[0:w, h, :]
            )
            w *= 2
        nc.vector.tensor_mul(
            out=sb_o[b0 : b0 + 64, t, :],
            in0=pso[0:64, :],
            in1=sb_R[0:64, h, :],
        )

    # ---- emission order == scheduling priority ----
    # Front-loaded projections keep the PE ramped; copy engines and queue
    # positions are chosen so every PSUM rotation wait lands on a copy that
    # is already done, and rope stat ops never sit ahead of softmax exps in
    # the ACT FIFO.
    qk_block(0, ppm, "dve")
    qk_block(4, ppm, "act")
    rope_core(0)
    qk_block(1, ppm, "act")
    qk_block(5, ppm, "act")
    rope_core(1)
    for jb in range(8):
        v_block(jb)
    qk_block(2, ppo, "act")
    qk_block(6, ppo, "act")
    rope_finish(0)
    qk_block(3, ppo, "dve")
    qk_block(7, ppo, "dve")
    ets0 = attn_sims(0)
    rope_finish(1)
    rope_core(2)
    load_weights_late()
    ets1 = attn_sims(1)
    rope_finish(2)
    rope_core(3)
    attn_avs(0, ets0)
    prev = ets1
    cur = attn_sims(2)
    attn_avs(1, prev)
    prev = cur
    rope_finish(3)
    for h in range(3, HEADS):
        cur = attn_sims(h)
        attn_avs(h - 1, prev)
        prev = cur
    attn_avs(HEADS - 1, prev)

    # ---- output projection (bias folded in as a K=1 ones-row matmul) ----
    psf = [ppm.tile([128, S], f32, tag="mm", name=f"psf{ob}") for ob in range(2)]
    for ob in range(2):
        for nn in range(2):
            nc.tensor.matmul(
                psf[ob][:, nn * 512 : (nn + 1) * 512],
                lhsT=(sb_bo[:, ob * 128 : (ob + 1) * 128]),
                rhs=(sb_one[:, nn * 512 : (nn + 1) * 512]),
                start=True,
                stop=False,
            )
    for kk in range(4):
        for ob in range(2):
            for nn in range(2):
                nc.tensor.matmul(
                    psf[ob][:, nn * 512 : (nn + 1) * 512],
                    lhsT=(sb_wo[:, kk, ob * 128 : (ob + 1) * 128]),
                    rhs=(sb_o[:, kk, nn * 512 : (nn + 1) * 512]),
                    start=False,
                    stop=(kk == 3),
                )
    # DMA can't read PSUM: stage through SBUF, one copy per engine so the
    # two output halves drain in parallel
    sb_out = [
        singles.tile([128, S], bf16, name=f"sb_out{ob}") for ob in range(2)
    ]
    nc.scalar.copy(out=sb_out[0][:, :], in_=psf[0][:, :])
    nc.vector.tensor_copy(out=sb_out[1][:, :], in_=psf[1][:, :])
    for ob in range(2):
        nc.sync.dma_start(out=out_d[ob * 128 : (ob + 1) * 128, :], in_=sb_out[ob][:, :])


# revision 30
# speedup vs baseline: 1.5023x; 1.5023x over previous
"""Trainium2 Bass kernel for AttentionWithRotaryPosEmb (8 cores, data-parallel).

Strategy
--------
Data-parallel over batch: each of the 8 NeuronCores computes one batch element
end-to-end. No collectives needed.

Per-core pipeline (batch x_b is [C=256, S=1024]):
  1. QKV projection:  q,k as [o, s] (o = 64h+d), vT as [s, hid] -- the vT
     orientation comes straight out of the matmul (lhsT=x, rhs=w_v^T), so no
     transposes are ever needed on-chip.  PSUM->SBUF casts run on the (idle)
     Pool/GPSIMD engine so DVE stays free for rope math.
  2. RoPE on q,k in bf16 [d, s] layout (DVE 2x packed mode): rotate-half is a
     partition-shifted DMA copy into a scratch tile, then sin/cos multiplies
     against fp16 row tables and one add.
  3. L2 norm over the sequence axis: fused square+reduce on DVE
     (tensor_tensor_reduce), rsqrt via exp(-0.5*ln(x)) on ACT, both q and k
     scales folded into q with one tensor_scalar multiply.
  4. Attention with *transposed* softmax: simT[j, i] blocks via
     matmul(lhsT=k_h, rhs=q_h); exp on ACT with scale=10 applied for free; the
     softmax denominators come for free from a ones-column appended to vT
     (65th lhsT column); outT[d, i] accumulates over j-blocks in PSUM.
     No max-subtraction needed: |10*sim| <= ~1 by construction (l2-normalized).
  5. Normalize via gpsimd partition_broadcast of the denominator row + one
     fused DVE divide (PSUM -> bf16), then the output projection and bias.

Engine budget per core (cost-model ns): PE 68.2us (320 matmuls), ACT ~67us
(64 softmax exps), DVE ~46us, Pool ~31us, all overlapped.
"""

import sys

import numpy as np

if "/opt/trn_rl_repo" not in sys.path:
    sys.path.insert(0, "/opt/trn_rl_repo")

HEADS = 8
DH = 64
S = 1024
C = 256
HID = 512
ROT = 32
HALF = 16
SCALE = 10.0
N_CORES = 8

_CACHE = {}


def _rope_tables():
    """Row-patterned cos/sin tables [128, S] matching the q/k SBUF layout.

    Partition row r holds o-row (128t + r) of q/k tile t, i.e. head 2t + (r>=64)
    and d = r % 64.  Rows d in [0,16) get cos(i*invf[d]) / -sin(i*invf[d]);
    rows d in [16,32) get cos(i*invf[d-16]) / +sin(i*invf[d-16]); rows d >= 32
    get cos=1, sin=0 (identity).  The sign of sin encodes rotate_half.
    fp16 keeps the DVE multiplies in 2x packed mode (f32 tables would drop
    the whole rope pipeline to full-rate).
    """
    inv = (
        1.0 / (np.float32(10000.0) ** (np.arange(0, ROT, 2, dtype=np.float32) / np.float32(ROT)))
    ).astype(np.float32)
    ang = (np.arange(S, dtype=np.float32)[None, :] * inv[:, None]).astype(np.float32)
    cos16 = np.cos(ang).astype(np.float32)  # [16, S]
    sin16 = np.sin(ang).astype(np.float32)
    cosT = np.ones((128, S), np.float32)
    sinT = np.zeros((128, S), np.float32)
    for r in range(128):
        d = r % 64
        if d < HALF:
            cosT[r] = cos16[d]
            sinT[r] = -sin16[d]
        elif d < ROT:
            cosT[r] = cos16[d - HALF]
            sinT[r] = sin16[d - HALF]
    return cosT.astype(np.float16), sinT.astype(np.float16)


def _emit(ctx, tc, aps):
    import concourse.bass as bass  # noqa: F401
    from concourse import mybir

    f32 = mybir.dt.float32
    bf16 = mybir.dt.bfloat16
    f16 = mybir.dt.float16
    AF = mybir.ActivationFunctionType
    ALU = mybir.AluOpType
    nc = tc.nc
    x_d, wq_d, wo_d, bo_d, cos_d, sin_d, zer_d, out_d = aps

    singles = ctx.enter_context(tc.tile_pool(name="singles", bufs=1))
    etp = ctx.enter_context(tc.tile_pool(name="etp", bufs=12))
    ppm = ctx.enter_context(tc.tile_pool(name="ppm", bufs=2, space="PSUM"))
    ppo = ctx.enter_context(tc.tile_pool(name="ppo", bufs=2, space="PSUM"))

    # ---- persistent SBUF ----
    # matmul operands are bf16 (rounded once, late); rope math in bf16 too
    # (DVE 2x packed mode), norm stats and softmax denominators in f32
    sb_x = singles.tile([128, 2, S], bf16)
    sb_wq = singles.tile([128, 2, 3 * HID], bf16)
    sb_wo = singles.tile([128, 4, C], bf16)
    sb_bo = singles.tile([1, 256], bf16)
    sb_one = singles.tile([1, S], bf16)
    sb_cos = singles.tile([128, S], f16)
    sb_sin = singles.tile([128, S], f16)
    sb_qr = singles.tile([128, 4, S], bf16)   # raw q (pre-rope), from PSUM
    sb_kr = singles.tile([128, 4, S], bf16)
    sb_tq = singles.tile([128, 4, S], bf16)   # rotate-half scratch
    sb_tk = singles.tile([128, 4, S], bf16)
    sb_qb = singles.tile([128, 4, S], bf16)   # roped+normalized q (matmul op)
    sb_kb = singles.tile([128, 4, S], bf16)   # roped k (matmul operand)
    sb_vx = singles.tile([128, 8, HEADS * (DH + 1)], bf16)
    sb_R = singles.tile([64, 8, S], f32)      # broadcast softmax denominators
    sb_rc = singles.tile([1, 8, S], f32)      # per-head reciprocal rows
    sb_o = singles.tile([128, 4, S], bf16)    # attention out (matmul operand)
    sb_stat = singles.tile([128, 16], f32)

    # ---- input loads (c-block granularity so QKV matmuls can start early) ----
    # SP queue carries only the startup-critical loads; wo/bias/zeros go on
    # the (idle-at-start) ACT queue so they don't delay the rope rotate DMAs.
    nc.sync.dma_start(out=sb_wq[:, 0, :], in_=wq_d[0:128, :])
    nc.sync.dma_start(out=sb_x[:, 0, :], in_=x_d[0:128, :])
    nc.sync.dma_start(out=sb_wq[:, 1, :], in_=wq_d[128:256, :])
    nc.sync.dma_start(out=sb_x[:, 1, :], in_=x_d[128:256, :])
    nc.sync.dma_start(out=sb_cos, in_=cos_d[:, :])
    nc.sync.dma_start(out=sb_sin, in_=sin_d[:, :])
    nc.vector.memset(sb_one[:, :], 1.0)

    def load_weights_late():
        """wo/bias loads, emitted after the first exp burst: they are only
        needed ~90us in, and issuing them at t=0 would steal the HWDGE
        (one shared 625ns/DMA generator) from the startup-critical loads."""
        for kk in range(4):
            nc.scalar.dma_start(
                out=sb_wo[:, kk, :], in_=wo_d[kk * 128 : (kk + 1) * 128, :]
            )
        nc.scalar.dma_start(out=sb_bo[:, :], in_=bo_d[:, :])

    # ones column per head in vT_ext -> free softmax denominators
    vx4 = sb_vx.rearrange("p j (h e) -> p j h e", e=DH + 1)
    nc.vector.memset(vx4[:, :, :, DH : DH + 1], 1.0)

    def copy_ps(eng, out, in_):
        """PSUM f32 -> SBUF bf16 cast on the named engine queue."""
        if eng == "act":
            nc.scalar.copy(out=out, in_=in_)
        elif eng == "dve":
            nc.vector.tensor_copy(out=out, in_=in_)
        else:
            nc.gpsimd.tensor_copy(out=out, in_=in_)

    def qk_block(ob, pool, cp):
        """QKV matmul for q (ob<4) / k (ob>=4) o-block -> SBUF bf16 cast.

        `pool` picks the PSUM rotation (ppm shares the sim-block tag, ppo
        the v/attention-out tag) and `cp` the copy-engine queue; both are
        chosen per block so no pool rotation ever waits on a slow copy."""
        tag = "mm" if pool is ppm else "ov"
        ps = pool.tile([128, S], f32, tag=tag, name=f"ps_qk{ob}")
        for nn in range(2):
            for kk in range(2):
                nc.tensor.matmul(
                    ps[:, nn * 512 : (nn + 1) * 512],
                    lhsT=(sb_wq[:, kk, ob * 128 : (ob + 1) * 128]),
                    rhs=(sb_x[:, kk, nn * 512 : (nn + 1) * 512]),
                    start=(kk == 0),
                    stop=(kk == 1),
                )
        dst3 = sb_qr if ob < 4 else sb_kr
        copy_ps(cp, dst3[:, ob % 4, :], ps[:, :])

    def rope_core(t):
        """RoPE for q/k tile t: rotate-half swaps + sin/cos multiply-add and
        the fused sum-of-squares.  q-side swaps ride the SP hardware DGE,
        k-side swaps run as Pool-engine software DGE so the two dispatch
        chains proceed in parallel.  DVE ops are interleaved q/k so neither
        side's rotate latency blocks the other's cos multiply."""
        q, k = sb_qr[:, t, :], sb_kr[:, t, :]
        for src3, tmp3, dq in ((sb_qr, sb_tq, nc.sync), (sb_kr, sb_tk, nc.gpsimd)):
            for base in (0, 64):
                dq.dma_start(
                    out=tmp3[base : base + 16, t, :],
                    in_=src3[base + 16 : base + 32, t, :],
                )
                dq.dma_start(
                    out=tmp3[base + 16 : base + 32, t, :],
                    in_=src3[base : base + 16, t, :],
                )
        tq, tk = sb_tq[:, t, :], sb_tk[:, t, :]
        nc.vector.tensor_mul(out=q, in0=q, in1=sb_cos[:, :])
        nc.vector.tensor_mul(out=k, in0=k, in1=sb_cos[:, :])
        # sign folded into sinT; its zero rows kill the d>=32 garbage
        nc.vector.tensor_mul(out=tq, in0=tq, in1=sb_sin[:, :])
        nc.vector.tensor_mul(out=tk, in0=tk, in1=sb_sin[:, :])
        nc.vector.tensor_add(out=sb_qb[:, t, :], in0=q, in1=tq)
        nc.vector.tensor_add(out=sb_kb[:, t, :], in0=k, in1=tk)
        for dst3, tmp3, col in ((sb_qb, sb_tq, 0), (sb_kb, sb_tk, 4)):
            # fused sum-of-squares over s; the full-width product is dumped
            # into the (now dead) rotate scratch
            nc.vector.tensor_tensor_reduce(
                out=tmp3[:, t, :],
                in0=dst3[:, t, :],
                in1=dst3[:, t, :],
                scale=1.0,
                scalar=0.0,
                op0=ALU.mult,
                op1=ALU.add,
                accum_out=sb_stat[:, col + t : col + t + 1],
            )

    def rope_finish(t):
        """rs_comb = (ssq_q*ssq_k)^-1/2 on ACT, folded into q on DVE."""
        nc.scalar.mul(
            out=sb_stat[:, 8 + t : 9 + t],
            in_=sb_stat[:, t : t + 1],
            mul=sb_stat[:, 4 + t : 5 + t],
        )
        nc.scalar.activation(
            out=sb_stat[:, 8 + t : 9 + t], in_=sb_stat[:, 8 + t : 9 + t],
            func=AF.Ln, bias=0.0,
        )
        nc.scalar.activation(
            out=sb_stat[:, 12 + t : 13 + t], in_=sb_stat[:, 8 + t : 9 + t],
            func=AF.Exp, scale=-0.5,
        )
        nc.vector.tensor_scalar_mul(
            out=sb_qb[:, t, :], in0=sb_qb[:, t, :],
            scalar1=sb_stat[:, 12 + t : 13 + t],
        )

    def v_block(jb):
        """vT s-block: [s_jb, hid] straight from matmul, strided into vx ext."""
        psv = ppo.tile([128, 512], f32, tag="ov", name=f"psv{jb}")
        for kk in range(2):
            nc.tensor.matmul(
                psv[:, :],
                lhsT=(sb_x[:, kk, jb * 128 : (jb + 1) * 128]),
                rhs=(sb_wq[:, kk, 2 * HID : 3 * HID]),
                start=(kk == 0),
                stop=(kk == 1),
            )
        dst = sb_vx[:, jb, :].rearrange("p (h e) -> p h e", e=DH + 1)[:, :, 0:DH]
        src = psv.rearrange("p (h d) -> p h d", d=DH)
        # f32 PSUM -> bf16 SBUF cast; Pool cannot read PSUM, so these all
        # ride the (pre-softmax idle) ACT queue
        copy_ps("act", dst, src)

    def attn_sims(h):
        """sim blocks + exps for head h; returns the et tiles for attn_avs."""
        t, half = h // 2, h % 2
        b0 = 64 * half
        q_h = sb_qb[b0 : b0 + 64, t, :]
        k_h = sb_kb[b0 : b0 + 64, t, :]
        ets = []
        for jb in range(8):
            pss = ppm.tile([128, S], f32, tag="mm", name=f"pss{h}_{jb}")
            for nn in range(2):
                nc.tensor.matmul(
                    pss[:, nn * 512 : (nn + 1) * 512],
                    lhsT=(k_h[:, jb * 128 : (jb + 1) * 128]),
                    rhs=(q_h[:, nn * 512 : (nn + 1) * 512]),
                    start=True,
                    stop=True,
                )
            et = etp.tile([128, S], bf16, tag="et", name=f"et{h}_{jb}", bufs=12)
            nc.scalar.activation(out=et[:, :], in_=pss[:, :], func=AF.Exp, scale=SCALE)
            ets.append(et)
        return ets

    def attn_avs(h, ets):
        """av accumulation + softmax normalization for head h."""
        t, half = h // 2, h % 2
        b0 = 64 * half
        pso = ppo.tile([65, S], f32, tag="ov", name=f"pso{h}")
        for jb in range(8):
            for nn in range(2):
                nc.tensor.matmul(
                    pso[:, nn * 512 : (nn + 1) * 512],
                    lhsT=(sb_vx[:, jb, h * (DH + 1) : (h + 1) * (DH + 1)]),
                    rhs=(ets[jb][:, nn * 512 : (nn + 1) * 512]),
                    start=(jb == 0),
                    stop=(jb == 7),
                )
        # row 64 of pso = sum_j exp(sim) per i.  Pool cannot read PSUM, so:
        # reciprocal on DVE (PSUM -> SBUF row), partition-broadcast of the
        # SBUF row on Pool, then one DVE multiply normalizes the head.
        # The broadcast output stays at base partition 0 (per-head slots):
        # GPSIMD writes to a nonzero base partition are broken on HW.
        rcp = sb_rc[0:1, h, :]
        nc.vector.reciprocal(out=rcp, in_=pso[64:65, :])
        nc.gpsimd.partition_broadcast(sb_R[0:64, h, :], rcp)
        nc.vector.tensor_mul(
            out=sb_o[b0 : b0 + 64, t, :],
            in0=pso[0:64, :],
            in1=sb_R[0:64, h, :],
        )

    # ---- emission order == scheduling priority ----
    # Front-loaded projections keep the PE ramped; copy engines and queue
    # positions are chosen so every PSUM rotation wait lands on a copy that
    # is already done, and rope stat ops never sit ahead of softmax exps in
    # the ACT FIFO.
    qk_block(0, ppm, "dve")
    qk_block(4, ppm, "act")
    rope_core(0)
    qk_block(1, ppm, "act")
    qk_block(5, ppm, "act")
    rope_core(1)
    for jb in range(8):
        v_block(jb)
    qk_block(2, ppo, "act")
    qk_block(6, ppo, "act")
    rope_finish(0)
    qk_block(3, ppo, "dve")
    qk_block(7, ppo, "dve")
    ets0 = attn_sims(0)
    rope_finish(1)
    rope_core(2)
    load_weights_late()
    ets1 = attn_sims(1)
    rope_finish(2)
    rope_core(3)
    attn_avs(0, ets0)
    prev = ets1
    cur = attn_sims(2)
    attn_avs(1, prev)
    prev = cur
    rope_finish(3)
    for h in range(3, HEADS):
        cur = attn_sims(h)
        attn_avs(h - 1, prev)
        prev = cur
    attn_avs(HEADS - 1, prev)

    # ---- output projection (bias folded in as a K=1 ones-row matmul) ----
    psf = [ppm.tile([128, S], f32, tag="mm", name=f"psf{ob}") for ob in range(2)]
    for ob in range(2):
        for nn in range(2):
            nc.tensor.matmul(
                psf[ob][:, nn * 512 : (nn + 1) * 512],
                lhsT=(sb_bo[:, ob * 128 : (ob + 1) * 128]),
                rhs=(sb_one[:, nn * 512 : (nn + 1) * 512]),
                start=True,
                stop=False,
            )
    for kk in range(4):
        for ob in range(2):
            for nn in range(2):
                nc.tensor.matmul(
                    psf[ob][:, nn * 512 : (nn + 1) * 512],
                    lhsT=(sb_wo[:, kk, ob * 128 : (ob + 1) * 128]),
                    rhs=(sb_o[:, kk, nn * 512 : (nn + 1) * 512]),
                    start=False,
                    stop=(kk == 3),
                )
    # DMA can't read PSUM: stage through SBUF, one copy per engine so the
    # two output halves drain in parallel
    sb_out = [
        singles.tile([128, S], bf16, name=f"sb_out{ob}") for ob in range(2)
    ]
    nc.scalar.copy(out=sb_out[0][:, :], in_=psf[0][:, :])
    nc.vector.tensor_copy(out=sb_out[1][:, :], in_=psf[1][:, :])
    for ob in range(2):
        nc.sync.dma_start(out=out_d[ob * 128 : (ob + 1) * 128, :], in_=sb_out[ob][:, :])
